# revision 6
# baseline (speedup 1.0000x reference)
"""Trainium2 Bass kernel for nn_AdaptiveMobiusLayer.

Algebraic restructuring (verified vs reference at 1.7e-7 rel in f32):
  The twist T is linear with T^2 = -I, and the coupling c is a per-token
  scalar, so each cycle is out <- (I + c_n T) out.  Consequences:
    * out_final = alpha * x + beta * T(x), with per-token scalars
      alpha = 1 - (c0c1 + c0c2 + c1c2), beta = (c0+c1+c2) - c0c1c2.
      out_1 / out_2 are never materialized.
    * With A_n = W1^T out_n and B_0 = W1^T T x = (twisted W1)^T x:
      A_1 = A_0 + c0*B_0;  A_2 = (1 - c0c1)*A_1 + c1(1 + c0^2)*B_0.
      Only TWO large K=1024 matmuls (A_0, B_0) instead of 3 L1 passes;
      cycles 1/2 reuse them with cheap elementwise updates in bf16.
  Sigmoids are computed as 0.5 + 0.5*tanh(x/2) so every activation
  (Gelu/Tanh/Identity) lives in the single `gelu_and_others` ACT table
  set -> zero table reloads.
  fp8 weights are scaled x16 on host (entries ~N(0, 1/32) would be
  subnormal in e4m3); the 1/16 is folded into the ACT scale operand.

Sharding: pure data parallel over tokens; core c takes 2048 consecutive
tokens.  Host ships x twice: bf16 (reduce + final combine) and fp8
(matmul moving operand), both feature-major [1024, 2048].  One tiny
pairwise AllReduce ([128,8] f32) provides the seq-mean global context.
"""

import dataclasses
import sys

sys.path.insert(0, "/opt/trn_rl_repo")

import numpy as np

B, S, DIM = 4, 4096, 1024
NCORES = 8
TOK = B * S // NCORES  # 2048 tokens per core
CHUNK = 512
NCHUNK = TOK // CHUNK  # 4
WS = 16.0  # host-side fp8 weight scale; 1/WS folded into ACT scale

# twist: quarters [or, oi, pr, pi] -> [pr, -pi, -or, oi]; slab partner t+4
TWIST_SIGN = [+1, +1, -1, -1, -1, -1, +1, +1]

_CACHE = {}


def _build_graph():
    import concourse.bass as bass
    import concourse.bacc as bacc
    import concourse.tile as tile
    import concourse.mybir as mybir

    f32 = mybir.dt.float32
    bf16 = mybir.dt.bfloat16
    f8 = mybir.dt.float8e4
    AF = mybir.ActivationFunctionType
    ALU = mybir.AluOpType
    AX = mybir.AxisListType
    DR = mybir.MatmulPerfMode.DoubleRow

    nc = bacc.Bacc(
        "TRN2", target_bir_lowering=False, debug=False, num_devices=NCORES
    )

    # ---- DRAM parameters (per-core shard; layouts prepared on host) ----
    xb_d = nc.declare_dram_parameter("xb", [DIM, TOK], bf16, isOutput=False)
    x8_d = nc.declare_dram_parameter("x8", [DIM, TOK], f8, isOutput=False)
    # fp8 weights (x16), fo-major DoubleRow packing:
    # row fo*128+p, col s*128+j == w[s*128+p, fo*128+j]
    w1_d = nc.declare_dram_parameter("w1", [8 * 128, DIM], f8, isOutput=False)
    w1t_d = nc.declare_dram_parameter("w1t", [8 * 128, DIM], f8, isOutput=False)
    w2_d = nc.declare_dram_parameter("w2", [4 * 128, DIM], f8, isOutput=False)
    w3_d = nc.declare_dram_parameter("w3", [2 * 128, 512], f8, isOutput=False)
    w4_d = nc.declare_dram_parameter("w4", [128, 2, 1], f8, isOutput=False)
    # biases + scalars packed into one small tensor (single DMA):
    # cols 0-7 b1, 8-11 b2, 12-13 b3, 14-17 gb1, 18-19 gb2;
    # partition-0 scalars: [0,20]=b4/2 [0,21]=gb3/2 [0,22]=adaptive_range
    cst_d = nc.declare_dram_parameter("consts", [128, 23], f32, isOutput=False)
    gw1_d = nc.declare_dram_parameter("gw1", [DIM, 512], bf16, isOutput=False)
    gw2_d = nc.declare_dram_parameter("gw2", [512, 256], bf16, isOutput=False)
    gw3_d = nc.declare_dram_parameter("gw3", [256, 1], bf16, isOutput=False)
    out_d = nc.declare_dram_parameter("out", [DIM, TOK], bf16, isOutput=True)

    def bcastN(row_ap, n):
        """[128, 512] row AP -> [128, n(bcast), 512] stride-0 view."""
        v = row_ap.rearrange("p (o j) -> p o j", o=1)
        ap2 = [list(x) for x in v.ap]
        ap2[1] = [0, n]
        return dataclasses.replace(v, ap=type(v.ap)(ap2))

    with tile.TileContext(nc) as tc:
        with (
            tc.tile_pool(name="const", bufs=1) as const,
            tc.tile_pool(name="abx", bufs=1) as abx,
            tc.tile_pool(name="x8p", bufs=2) as x8p,
            tc.tile_pool(name="work", bufs=2) as work,
            tc.tile_pool(name="cv", bufs=1) as cv,
            tc.tile_pool(name="psL", bufs=2, space="PSUM") as psL,
            tc.tile_pool(name="psM", bufs=1, space="PSUM") as psM,
            tc.tile_pool(name="psx", bufs=2, space="PSUM") as psx,
            tc.tile_pool(name="dram", bufs=1, space="DRAM") as dram,
        ):
            # ---------------- DMA helpers ----------------
            # dma_start ISSUE costs ~0.6us on a sequencer; only sync has the
            # fast HWDGE path for bulk.  The ACT sequencer is idle early, so
            # it issues the small const/weight DMAs, letting sync start on x.
            early = [0]

            def dma_rr(out, in_):
                if early[0] > 0:
                    early[0] -= 1
                    nc.scalar.dma_start(out=out, in_=in_)
                else:
                    nc.sync.dma_start(out=out, in_=in_)

            def load_w_fo(dparam, kin, nfo, tagp):
                """[128, nk, 128] fp8 tiles (k = s*128 + p), one per fo."""
                tiles = []
                nk = kin // 128
                for fo in range(nfo):
                    t = const.tile([128, nk, 128], f8, tag=f"{tagp}_{fo}")
                    dma_rr(t[:], dparam[fo * 128:(fo + 1) * 128, :].rearrange(
                        "p (s j) -> p s j", s=nk))
                    tiles.append(t)
                return tiles

            def load_w(dparam, kin, nout, tagp):
                tiles = []
                for k in range(kin // 128):
                    t = const.tile([128, nout], bf16, tag=f"{tagp}_{k}")
                    dma_rr(t[:], dparam[k * 128:(k + 1) * 128, :])
                    tiles.append(t)
                return tiles

            early[0] = 9  # cst + the 8 w1a tiles on the ACT sequencer
            cst = const.tile([128, 23], f32, tag="cst")
            dma_rr(cst[:], cst_d[:, :])
            b1 = cst[:, 0:8]
            b2 = cst[:, 8:12]
            b3 = cst[:, 12:14]
            gb1 = cst[:, 14:18]
            gb2 = cst[:, 18:20]
            b4h = cst[0:1, 20:21]
            gb3h = cst[0:1, 21:22]
            ar = cst[0:1, 22:23]
            ones = const.tile([1, 128], bf16, tag="ones")
            nc.vector.memset(ones[:], 1.0)

            w1a = load_w_fo(w1_d, DIM, 8, "w1a")

            def load_x8(c):
                t = x8p.tile([128, 8, CHUNK], f8, tag="x8")
                dma_rr(t[:], x8_d[:, c * CHUNK:(c + 1) * CHUNK].rearrange(
                    "(s p) j -> p s j", p=128))
                return t

            def load_xbf(c):
                t = abx.tile([128, 8, CHUNK], bf16, tag=f"xbf_{c}")
                dma_rr(t[:], xb_d[:, c * CHUNK:(c + 1) * CHUNK].rearrange(
                    "(s p) j -> p s j", p=128))
                return t

            x8t = [None] * NCHUNK
            x8t[0] = load_x8(0)
            w1b = load_w_fo(w1t_d, DIM, 8, "w1b")
            x8t[1] = load_x8(1)
            xbf = [None] * NCHUNK
            xbf[0] = load_xbf(0)
            xbf[1] = load_xbf(1)
            w2f = load_w_fo(w2_d, DIM, 4, "w2f")
            w3f = load_w_fo(w3_d, 512, 2, "w3f")
            w4f = const.tile([128, 2, 1], f8, tag="w4f")
            dma_rr(w4f[:], w4_d[:, :, :])
            x8t[2] = load_x8(2)
            x8t[3] = load_x8(3)
            xbf[2] = load_xbf(2)
            xbf[3] = load_xbf(3)
            gw1 = load_w(gw1_d, DIM, 512, "gw1")
            gw2 = load_w(gw2_d, 512, 256, "gw2")
            gw3 = load_w(gw3_d, 256, 1, "gw3")

            # persistent A / B (bf16, x16 domain), per-chunk [128, 8, 512]
            At = [abx.tile([128, 8, CHUNK], bf16, tag=f"a_{c}", name=f"a_{c}")
                  for c in range(NCHUNK)]
            Bt = [abx.tile([128, 8, CHUNK], bf16, tag=f"b_{c}", name=f"b_{c}")
                  for c in range(NCHUNK)]

            # ---------------- L1: A0 / B0 matmuls + drains ----------------
            def a0b0_chunk(c, wtiles, dest):
                """8 output slabs of W^T x for chunk c -> dest bf16 tile."""
                for fop in range(4):
                    ps = psL.tile([128, 1024], f32, tag="l1")
                    for fo2 in range(2):
                        fo = fop * 2 + fo2
                        for s in range(4):
                            nc.tensor.matmul(
                                ps[:, fo2 * 512:(fo2 + 1) * 512],
                                wtiles[fo][:, 2 * s:2 * s + 2, :],
                                x8t[c][:, 2 * s:2 * s + 2, :],
                                start=(s == 0), stop=(s == 3), perf_mode=DR,
                            )
                    nc.vector.tensor_copy(
                        dest[:, 2 * fop:2 * fop + 2, :], ps[:])

            # ---------------- global-context partial sums + AllReduce ------
            red = const.tile([128, 8, NCHUNK], f32, tag="gred")

            def reduce_chunk(c):
                nc.vector.tensor_reduce(
                    red[:, :, c:c + 1], xbf[c][:], axis=AX.X, op=ALU.add)

            gs = const.tile([128, 8], f32, tag="gs")

            def finish_gsum():
                nc.vector.tensor_reduce(
                    gs[:].rearrange("p (t o) -> p t o", o=1),
                    red[:], axis=AX.X, op=ALU.add)

            cc_in = dram.tile([128, 8], f32, tag="cc_in")
            cc_out = dram.tile([128, 8], f32, tag="cc_out")
            gmean_f = const.tile([128, 8], f32, tag="gmean_f")
            gmean = const.tile([128, 8], bf16, tag="gmean")

            def do_collective():
                nc.sync.dma_start(out=cc_in[:], in_=gs[:])
                nc.gpsimd.collective_compute(
                    "AllReduce",
                    ALU.add,
                    ins=[cc_in.opt()],
                    outs=[cc_out.opt()],
                    replica_groups=[[0, 1], [2, 3], [4, 5], [6, 7]],
                )
                nc.sync.dma_start(out=gmean_f[:], in_=cc_out[:])
                nc.vector.tensor_copy(gmean[:], gmean_f[:])

            # ---------------- global net ----------------
            gc_t = {}

            def gc_stage1():
                ps = psx.tile([128, 4], f32, tag="aux")
                for fo in range(4):
                    for k in range(8):
                        nc.tensor.matmul(
                            ps[:, fo:fo + 1], gw1[k][:, fo * 128:(fo + 1) * 128],
                            gmean[:, k:k + 1], start=(k == 0), stop=(k == 7),
                        )
                z1 = work.tile([128, 4], f32, tag="z1")
                nc.vector.scalar_tensor_tensor(
                    z1[:], ps[:], 1.0 / S, gb1, ALU.mult, ALU.add)
                g1 = work.tile([128, 4], bf16, tag="g1")
                nc.scalar.activation(g1[:], z1[:], AF.Gelu)
                gc_t["g1"] = g1

            def gc_stage2():
                g1 = gc_t["g1"]
                ps = psx.tile([128, 2], f32, tag="aux")
                for fo in range(2):
                    for k in range(4):
                        nc.tensor.matmul(
                            ps[:, fo:fo + 1], gw2[k][:, fo * 128:(fo + 1) * 128],
                            g1[:, k:k + 1], start=(k == 0), stop=(k == 3),
                        )
                z2 = work.tile([128, 2], f32, tag="z2")
                nc.vector.tensor_add(z2[:], ps[:], gb2)
                g2 = work.tile([128, 2], bf16, tag="g2")
                nc.scalar.activation(g2[:], z2[:], AF.Gelu)
                gc_t["g2"] = g2

            def gc_stage3():
                g2 = gc_t["g2"]
                ps = psx.tile([1, 1], f32, tag="aux")
                for k in range(2):
                    nc.tensor.matmul(
                        ps[:], gw3[k][:, 0:1], g2[:, k:k + 1],
                        start=(k == 0), stop=(k == 1))
                # gf = sigmoid(z+gb3) = 0.5 + 0.5*tanh(z/2 + gb3/2)
                tg = const.tile([1, 1], f32, tag="tg")
                nc.scalar.activation(tg[:], ps[:], AF.Tanh, bias=gb3h, scale=0.5)
                # c = 0.1 + 0.7*ar*tg + 0.3*ar*t4  =: c0p + cmulp * t4
                cmulp = const.tile([1, 1], f32, tag="cmulp")
                nc.vector.tensor_scalar(cmulp[:], ar, 0.3, None, ALU.mult)
                t0 = const.tile([1, 1], f32, tag="t0")
                nc.vector.tensor_tensor(t0[:], tg[:], ar, ALU.mult)
                c0p = const.tile([1, 1], f32, tag="c0p")
                nc.vector.tensor_scalar(c0p[:], t0[:], 0.7, 0.1, ALU.mult, ALU.add)
                gc_t["cmulp"] = cmulp
                gc_t["c0p"] = c0p

            # ---------------- coupling-net MLP (cycles share this) --------
            t4s = [[None] * NCHUNK for _ in range(3)]

            def mlp_chunk(c, n):
                """L2..L4 + tanh on GELU(A_n) for chunk c; stores t4s[n][c]."""
                h1 = work.tile([128, 8, CHUNK], f8, tag="h1")
                for s in range(8):
                    nc.scalar.activation(
                        h1[:, s, :], At[c][:, s, :], AF.Gelu,
                        bias=b1[:, s:s + 1], scale=1.0 / WS)
                h2 = work.tile([128, 4, CHUNK], f8, tag="h2")
                for fop in range(2):
                    ps = psM.tile([128, 1024], f32, tag="mlp")
                    for fo2 in range(2):
                        fo = fop * 2 + fo2
                        for s in range(4):
                            nc.tensor.matmul(
                                ps[:, fo2 * 512:(fo2 + 1) * 512],
                                w2f[fo][:, 2 * s:2 * s + 2, :],
                                h1[:, 2 * s:2 * s + 2, :],
                                start=(s == 0), stop=(s == 3), perf_mode=DR,
                            )
                    for fo2 in range(2):
                        fo = fop * 2 + fo2
                        nc.scalar.activation(
                            h2[:, fo, :], ps[:, fo2 * 512:(fo2 + 1) * 512],
                            AF.Gelu, bias=b2[:, fo:fo + 1], scale=1.0 / WS)
                h3 = work.tile([128, 2, CHUNK], f8, tag="h3")
                ps3 = psM.tile([128, 1024], f32, tag="mlp")
                for fo in range(2):
                    for s in range(2):
                        nc.tensor.matmul(
                            ps3[:, fo * 512:(fo + 1) * 512],
                            w3f[fo][:, 2 * s:2 * s + 2, :],
                            h2[:, 2 * s:2 * s + 2, :],
                            start=(s == 0), stop=(s == 1), perf_mode=DR,
                        )
                for fo in range(2):
                    nc.scalar.activation(
                        h3[:, fo, :], ps3[:, fo * 512:(fo + 1) * 512],
                        AF.Gelu, bias=b3[:, fo:fo + 1], scale=1.0 / WS)
                ps4 = psx.tile([1, CHUNK], f32, tag="aux")
                for s in range(2):
                    nc.tensor.matmul(
                        ps4[:], w4f[:, s, :], h3[:, s, :],
                        start=(s == 0), stop=(s == 1))
                # tf = sigmoid(z4+b4) -> t4 = tanh(z4/2 + b4/2); z4 is x16
                t4 = cv.tile([1, CHUNK], bf16, tag="t4", bufs=4)
                nc.scalar.activation(
                    t4[:], ps4[:], AF.Tanh, bias=b4h, scale=1.0 / (2.0 * WS))
                t4s[n][c] = t4
                return t4

            # ---------------- per-chunk coupling vectors ----------------
            cvecs = [[None] * NCHUNK for _ in range(3)]

            def make_cvec(c, n):
                t4 = t4s[n][c]
                cvec = cv.tile([1, CHUNK], bf16, tag=f"c_{n}_{c}")
                nc.scalar.activation(
                    cvec[:], t4[:], AF.Identity,
                    bias=gc_t["c0p"][:], scale=gc_t["cmulp"][:])
                cvecs[n][c] = cvec
                return cvec

            def bcast_row(vec, tag):
                """[1,512] bf16 -> [128,512] bf16 via K=1 matmul + ACT cast."""
                ps = psx.tile([128, CHUNK], f32, tag="aux")
                nc.tensor.matmul(ps[:], ones[:], vec[:], start=True, stop=True)
                row = work.tile([128, CHUNK], bf16, tag=tag, bufs=1)
                nc.scalar.activation(row[:], ps[:], AF.Identity)
                return row

            # ---------------- A updates ----------------
            def a1_update(c):
                cb = bcast_row(cvecs[0][c], "cb0")
                tmp = work.tile([128, 8, CHUNK], bf16, tag="abtmp", bufs=1)
                nc.vector.tensor_mul(tmp[:], Bt[c][:], bcastN(cb[:], 8))
                nc.vector.tensor_add(At[c][:], At[c][:], tmp[:])

            m01s = [None] * NCHUNK

            def coeff2(c):
                """p2 = 1-c0c1 ; q2 = c1*(1+c0^2) (vs A_1, B_0)."""
                c0, c1 = cvecs[0][c], cvecs[1][c]
                m01 = cv.tile([1, CHUNK], bf16, tag=f"m01_{c}", name=f"m01_{c}")
                nc.vector.tensor_mul(m01[:], c0[:], c1[:])
                m01s[c] = m01
                p2 = cv.tile([1, CHUNK], bf16, tag="p2", bufs=1)
                nc.vector.tensor_scalar(p2[:], m01[:], -1.0, 1.0, ALU.mult, ALU.add)
                c0sq = cv.tile([1, CHUNK], bf16, tag="c0sq", bufs=1)
                nc.vector.tensor_mul(c0sq[:], c0[:], c0[:])
                u = cv.tile([1, CHUNK], bf16, tag="u", bufs=1)
                nc.vector.tensor_scalar(u[:], c0sq[:], 1.0, 1.0, ALU.mult, ALU.add)
                q2 = cv.tile([1, CHUNK], bf16, tag="q2", bufs=1)
                nc.vector.tensor_mul(q2[:], c1[:], u[:])
                return p2, q2

            def a2_update(c, p2, q2):
                p2b = bcast_row(p2, "p2b")
                q2b = bcast_row(q2, "q2b")
                tmp = work.tile([128, 8, CHUNK], bf16, tag="abtmp", bufs=1)
                nc.vector.tensor_mul(tmp[:], Bt[c][:], bcastN(q2b[:], 8))
                nc.vector.tensor_mul(At[c][:], At[c][:], bcastN(p2b[:], 8))
                nc.vector.tensor_add(At[c][:], At[c][:], tmp[:])

            # ---------------- final combine + output ----------------
            def alphabeta(c):
                c0, c1, c2 = cvecs[0][c], cvecs[1][c], cvecs[2][c]
                m01 = m01s[c]
                s01 = cv.tile([1, CHUNK], bf16, tag="s01", bufs=1)
                nc.vector.tensor_add(s01[:], c0[:], c1[:])
                t = cv.tile([1, CHUNK], bf16, tag="tt", bufs=1)
                nc.vector.tensor_mul(t[:], c2[:], s01[:])
                u2 = cv.tile([1, CHUNK], bf16, tag="u2", bufs=1)
                nc.vector.tensor_add(u2[:], t[:], m01[:])
                alpha = cv.tile([1, CHUNK], bf16, tag="alpha", bufs=1)
                nc.vector.tensor_scalar(alpha[:], u2[:], -1.0, 1.0, ALU.mult, ALU.add)
                v = cv.tile([1, CHUNK], bf16, tag="v", bufs=1)
                nc.vector.tensor_scalar(v[:], m01[:], -1.0, 1.0, ALU.mult, ALU.add)
                w = cv.tile([1, CHUNK], bf16, tag="w", bufs=1)
                nc.vector.tensor_mul(w[:], c2[:], v[:])
                beta = cv.tile([1, CHUNK], bf16, tag="beta", bufs=1)
                nc.vector.tensor_add(beta[:], s01[:], w[:])
                return alpha, beta

            def combine_chunk(c, alpha, beta):
                ab = bcast_row(alpha, "abr")
                psb = psx.tile([128, CHUNK], f32, tag="aux")
                nc.tensor.matmul(psb[:], ones[:], beta[:], start=True, stop=True)
                bb = work.tile([128, CHUNK], bf16, tag="bbr", bufs=1)
                nc.scalar.activation(bb[:], psb[:], AF.Identity)
                bbn = work.tile([128, CHUNK], bf16, tag="bbn", bufs=1)
                nc.scalar.activation(bbn[:], psb[:], AF.Identity, scale=-1.0)
                # quarters: slabs {2q,2q+1} with partner {2q+4 mod 8}
                for q in range(4):
                    t0 = 2 * q
                    u0 = (2 * q + 4) % 8
                    bsel = bb if TWIST_SIGN[t0] > 0 else bbn
                    tmp = work.tile([128, 2, CHUNK], bf16, tag="ctmp")
                    nc.vector.tensor_mul(
                        tmp[:], xbf[c][:, u0:u0 + 2, :], bcastN(bsel[:], 2))
                    o = work.tile([128, 2, CHUNK], bf16, tag="outq")
                    nc.vector.tensor_mul(
                        o[:], xbf[c][:, t0:t0 + 2, :], bcastN(ab[:], 2))
                    nc.vector.tensor_add(o[:], o[:], tmp[:])
                    nc.sync.dma_start(
                        out=out_d[t0 * 128:(t0 + 2) * 128,
                                  c * CHUNK:(c + 1) * CHUNK].rearrange(
                            "(s p) j -> p s j", p=128),
                        in_=o[:])

            # ================= emission schedule =================
            # cycle 0: A0/B0 + MLP0 chunk-pipelined; reduces eager; gc-net
            # after the collective lands; cb0 broadcasts early so the DVE
            # update chain for chunk c completes long before the PE reaches
            # MLP1(c).
            a0b0_chunk(0, w1a, At[0])
            a0b0_chunk(0, w1b, Bt[0])
            reduce_chunk(0)
            a0b0_chunk(1, w1a, At[1])
            a0b0_chunk(1, w1b, Bt[1])
            reduce_chunk(1)
            mlp_chunk(0, 0)
            a0b0_chunk(2, w1a, At[2])
            a0b0_chunk(2, w1b, Bt[2])
            reduce_chunk(2)
            mlp_chunk(1, 0)
            reduce_chunk(3)
            finish_gsum()
            do_collective()
            a0b0_chunk(3, w1a, At[3])
            a0b0_chunk(3, w1b, Bt[3])
            gc_stage1()
            gc_stage2()
            gc_stage3()
            mlp_chunk(2, 0)
            make_cvec(0, 0)
            a1_update(0)
            mlp_chunk(3, 0)
            make_cvec(1, 0)
            a1_update(1)
            make_cvec(2, 0)
            a1_update(2)
            make_cvec(3, 0)
            a1_update(3)
            # cycle 1
            mlp_chunk(0, 1)
            mlp_chunk(1, 1)
            make_cvec(0, 1)
            a2_update(0, *coeff2(0))
            mlp_chunk(2, 1)
            make_cvec(1, 1)
            a2_update(1, *coeff2(1))
            mlp_chunk(3, 1)
            make_cvec(2, 1)
            a2_update(2, *coeff2(2))
            # cycle 2
            mlp_chunk(0, 2)
            make_cvec(3, 1)
            a2_update(3, *coeff2(3))
            mlp_chunk(1, 2)
            make_cvec(0, 2)
            combine_chunk(0, *alphabeta(0))
            mlp_chunk(2, 2)
            make_cvec(1, 2)
            combine_chunk(1, *alphabeta(1))
            mlp_chunk(3, 2)
            make_cvec(2, 2)
            combine_chunk(2, *alphabeta(2))
            make_cvec(3, 2)
            combine_chunk(3, *alphabeta(3))

    nc.compile()
    return nc


def _get_graph():
    if "nc" not in _CACHE:
        _CACHE["nc"] = _build_graph()
    return _CACHE["nc"]


def _pack_consts(inputs):
    cst = np.zeros((128, 23), np.float32)
    cst[:, 0:8] = np.asarray(inputs["cn_b1"], np.float32).reshape(8, 128).T
    cst[:, 8:12] = np.asarray(inputs["cn_b2"], np.float32).reshape(4, 128).T
    cst[:, 12:14] = np.asarray(inputs["cn_b3"], np.float32).reshape(2, 128).T
    cst[:, 14:18] = np.asarray(inputs["gc_b1"], np.float32).reshape(4, 128).T
    cst[:, 18:20] = np.asarray(inputs["gc_b2"], np.float32).reshape(2, 128).T
    cst[0, 20] = 0.5 * float(np.asarray(inputs["cn_b4"]).reshape(()))
    cst[0, 21] = 0.5 * float(np.asarray(inputs["gc_b3"]).reshape(()))
    cst[0, 22] = float(np.asarray(inputs["adaptive_range"]).reshape(()))
    return cst


def _pack_dr(w, kin, nfo):
    """[kin, nfo*128] -> fo-major DoubleRow layout [nfo*128, kin] (x16)."""
    ns = kin // 128
    return np.ascontiguousarray(
        (np.asarray(w, np.float32) * WS)
        .reshape(ns, 128, nfo, 128).transpose(2, 1, 0, 3).reshape(nfo * 128, kin))


def _make_in_maps(inputs):
    import ml_dtypes

    bf = ml_dtypes.bfloat16
    f8 = ml_dtypes.float8_e4m3

    x = np.ascontiguousarray(np.asarray(inputs["x"], np.float32))
    xs = x.reshape(NCORES, TOK, DIM).transpose(0, 2, 1)  # [8, 1024, 2048]

    W1 = np.asarray(inputs["cn_w1"], np.float32)
    # twisted W1: slab j of W1t = TWIST_SIGN[j+4 mod 8] * W1 slab (j+4 mod 8)
    W1t = np.concatenate(
        [TWIST_SIGN[(j + 4) % 8] * W1[((j + 4) % 8) * 128:((j + 4) % 8 + 1) * 128, :]
         for j in range(8)], axis=0)

    shared = {
        "w1": _pack_dr(W1, DIM, 8).astype(f8),
        "w1t": _pack_dr(W1t, DIM, 8).astype(f8),
        "w2": _pack_dr(inputs["cn_w2"], DIM, 4).astype(f8),
        "w3": _pack_dr(inputs["cn_w3"], 512, 2).astype(f8),
        "w4": np.ascontiguousarray(
            (np.asarray(inputs["cn_w4"], np.float32) * WS)
            .reshape(2, 128).T.reshape(128, 2, 1)).astype(f8),
        "gw1": np.ascontiguousarray(inputs["gc_w1"]).astype(bf),
        "gw2": np.ascontiguousarray(inputs["gc_w2"]).astype(bf),
        "gw3": np.ascontiguousarray(
            np.asarray(inputs["gc_w3"]).reshape(256, 1)).astype(bf),
        "consts": _pack_consts(inputs),
    }
    in_maps = []
    for c in range(NCORES):
        m = dict(shared)
        xc = np.ascontiguousarray(xs[c])
        m["xb"] = xc.astype(bf)
        m["x8"] = xc.astype(f8)
        in_maps.append(m)
    return in_maps


def _run(inputs, trace=False):
    from concourse.bass_utils import run_bass_kernel_spmd

    nc = _get_graph()
    in_maps = _make_in_maps(inputs)
    res = run_bass_kernel_spmd(
        nc, in_maps, core_ids=list(range(NCORES)), trace=trace
    )
    outs = np.stack(
        [np.asarray(res.results[c]["out"]).astype(np.float32).T
         for c in range(NCORES)], axis=0
    )  # [8, 2048, 1024]
    full = outs.reshape(B, S, DIM)
    return full, res


def kernel(**inputs) -> np.ndarray:
    out, _ = _run(inputs, trace=False)
    return out
